# revision 15
# baseline (speedup 1.0000x reference)
"""Trainium2 Bass kernel for the SNN (two-layer LIF, snnTorch-style) problem.

Math (per batch row b, fp32):
    cur1 = x @ W1.T + b1                       # [B, NH], constant across steps
    mem1_{t+1} = beta*mem1_t + cur1 - H(mem1_t - 1)      (mem1_1 = cur1)
    spk1_t  = H(mem1_t - 1)                    # == reset used at step t+1
    cur2_t  = spk1_t @ W2.T + b2
    mem2_t  = beta*mem2_{t-1} + cur2_t - H(mem2_{t-1} - 1)
    outputs: mem2_rec[t] = mem2_t, spk2_rec[t] = H(mem2_t - 1)

Key kernel ideas:
  * One fused custom DVE op per LIF step:  out = (in0*C0 + in1) - (in0 > C1).
    The reset spike is recomputed from mem inside the op, so the whole
    [128, 2048] state update is a single 1x DVE pass per tile.
  * spk1 is never materialized. Since s_t = beta*mem_t + cur1 - mem_{t+1}
    (exactly, up to fp32 rounding), W2 @ s_t = beta*r_t + q - r_{t+1} with
    r_t = W2 @ mem_t and q = W2 @ cur1 + b2. The PE matmuls the mem state
    directly each step; folding a = q/(beta-1) into r gives
    cur2_t = beta*r~_t - r~_{t+1} with r~ = r + a.
  * r [2, 2048] is PE-transposed into batch-major [128, 32] so the tiny mem2
    recurrence runs at full partition utilization, again with the fused op.

Data parallel over batch: 16384 rows -> 8 cores x 2048.
"""

import sys

if "/opt/trn_rl_repo" not in sys.path:
    sys.path.insert(0, "/opt/trn_rl_repo")

import numpy as np

import concourse.bacc as bacc
import concourse.bass as bass
import concourse.mybir as mybir
import concourse.tile as tile
from concourse.bass_utils import run_bass_kernel_spmd

# Problem constants (hardcoded; kernel.py must be self-contained).
B, NI, NH, NO, T = 16384, 100, 1000, 2, 25
NCORES = 8
BS = B // NCORES          # 2048 batch rows per core
NHP = 1024                # hidden padded to 8 * 128
NT = NHP // 128           # 8 hidden tiles
NBLK = BS // 128          # 16 batch blocks
NCH = BS // 512           # 4 moving chunks of 512
BETA = 0.95
THR = 1.0
F32 = mybir.dt.float32
AOP = mybir.AluOpType
AFT = mybir.ActivationFunctionType

_LIF_OP = None


def _get_lif_op():
    """Register (once) the fused LIF-step op: out = (in0*s0 + in1) - (in0 > s1)."""
    global _LIF_OP
    if _LIF_OP is not None:
        return _LIF_OP
    from concourse import dve_ops
    from concourse.dve_spec import Spec, Src0, Src1, C0, C1, lower, _has_src1
    from concourse.dve_uop import DveOpSpec

    name = "LIF_STEP_ANT"
    for op in dve_ops.OPS:
        if op.name == name:
            _LIF_OP = op
            return op

    spec = Spec(
        body=(Src0 * C0 + Src1) - (Src0 > C1),
        reference=lambda in0, in1, s0, s1, imm2: (
            in0.astype(np.float32) * s0 + in1
        )
        - (in0 > s1).astype(np.float32),
    )
    row = dve_ops._CUSTOM_DVE_ROW_BASE + len(dve_ops.OPS)
    assert row < 0x20, "custom-DVE row space exhausted"
    dve_ops._SUB_OPCODE_FOR_NAME[name] = row
    shas = {}
    for ver in ("v3", "v4"):
        s = DveOpSpec(
            name=name, opcode=row, uops=lower(spec, ver=ver), rd1_en=_has_src1(spec)
        )
        shas[ver] = s.sha(ver)
    op = dve_ops.DveOp(name, spec, subdim=False, uops_sha=shas)
    dve_ops.OPS.append(op)
    dve_ops.CUSTOM_DVE_SPECS[name] = spec
    _LIF_OP = op
    return op


# Packed-input layout: one DMA -> one completion semaphore, so matmuls never
# need more than one sync-wait (walrus allows only one per Matmult).
OFF_X = 0                      # [128, NBLK*NI] x as (p, blk, f)
OFF_ID = OFF_X + NBLK * NI     # [128, 128] identity
OFF_W1 = OFF_ID + 128          # [:NI, NHP] W1.T (padded)
OFF_B1 = OFF_W1 + NHP          # [128, NT] b1 per-tile columns
OFF_W2 = OFF_B1 + NT           # [128, NT*NO] W2.T tiles
OFF_B2 = OFF_W2 + NT * NO      # [:NO, 1] b2
BLOBF = OFF_B2 + 1


def _build_program():
    lif = _get_lif_op()
    a_scale = float(1.0 / (float(np.float32(BETA)) - 1.0))  # q -> a fold factor

    nc = bacc.Bacc(
        "TRN2",
        target_bir_lowering=False,
        debug=False,
        num_devices=NCORES,
    )
    blob_d = nc.declare_dram_parameter("blob", [128, BLOBF], F32, isOutput=False)
    m2_d = nc.declare_dram_parameter("mem2_rec", [T, BS, NO], F32, isOutput=True)
    s2_d = nc.declare_dram_parameter("spk2_rec", [T, BS, NO], F32, isOutput=True)

    with tile.TileContext(nc) as tc:
        with (
            tc.tile_pool(name="const", bufs=1) as constp,
            tc.tile_pool(name="state", bufs=1) as statep,
            tc.tile_pool(name="work", bufs=3) as workp,
        ):
            blob = constp.tile([128, BLOBF], F32)
            nc.sync.dma_start(blob[:], blob_d[:])
            ident = blob[:, OFF_ID : OFF_ID + 128]
            b1c = blob[:, OFF_B1 : OFF_B1 + NT]
            w2s = blob[:, OFF_W2 : OFF_W2 + NT * NO].rearrange(
                "p (i o) -> p i o", o=NO
            )
            b2c = blob[:NO, OFF_B2 : OFF_B2 + 1]
            w1t = blob[:NI, OFF_W1 : OFF_W1 + NHP]

            cur1 = statep.tile([128, NT, BS], F32)
            mem = statep.tile([128, NT, BS], F32)
            rbuf = statep.tile([128, T + 2, NBLK * NO], F32)  # r~_t, slots 1..T+1
            a_sb = constp.tile([128, NBLK * NO], F32)
            m2rec = statep.tile([128, T, NBLK * NO], F32)
            s2rec = statep.tile([128, T, NBLK * NO], F32)
            zer32 = constp.tile([128, NBLK * NO], F32)
            nc.vector.memset(zer32[:], 0.0)

            # One-time per-engine "touch" of the blob so the DMA-completion
            # wait is observed once per engine; later instructions then never
            # need a second sync-wait slot (walrus allows one per instruction).
            scr = constp.tile([1, 2], F32)
            nc.scalar.activation(scr[:, 0:1], blob[:1, 0:1], AFT.Copy)
            nc.vector.tensor_copy(scr[:, 1:2], blob[:1, 0:1])

            # ---- setup: x -> xT, cur1 = W1 @ x + b1 ----
            with tc.tile_pool(name="pss", bufs=2, space=bass.MemorySpace.PSUM) as pss:
                xin = blob[:, OFF_X : OFF_X + NBLK * NI].rearrange(
                    "p (blk f) -> p blk f", f=NI
                )
                xT = constp.tile([NI, BS], F32)
                for blk in range(NBLK):
                    px = pss.tile([NI, 128], F32, tag="px")
                    # transpose via regular matmul: xin_blk.T @ I  (walrus allows
                    # only one sync-wait per Matmult; transpose-mode is worse)
                    nc.tensor.matmul(px[:], xin[:, blk, :], ident)
                    nc.scalar.activation(
                        xT[:, blk * 128 : (blk + 1) * 128], px[:], AFT.Copy
                    )
                for i in range(NT):
                    for c in range(NCH):
                        pc = pss.tile([128, 512], F32, tag="pc")
                        nc.tensor.matmul(
                            pc[:],
                            w1t[:, i * 128 : (i + 1) * 128],
                            xT[:, c * 512 : (c + 1) * 512],
                        )
                        nc.scalar.activation(
                            cur1[:, i, c * 512 : (c + 1) * 512],
                            pc[:],
                            AFT.Identity,
                            bias=b1c[:, i : i + 1],
                        )

            with (
                tc.tile_pool(name="pracc", bufs=1, space=bass.MemorySpace.PSUM) as pra,
                tc.tile_pool(name="prt", bufs=2, space=bass.MemorySpace.PSUM) as prtp,
            ):

                def r_matmul(src):
                    """psum[c] += W2T_i.T @ src[:, i, chunk c] over i -> [NO, 512] x4"""
                    prs = [
                        pra.tile([NO, 512], F32, tag=f"pr{c}", name=f"pr{c}")
                        for c in range(NCH)
                    ]
                    for c in range(NCH):
                        for i in range(NT):
                            nc.tensor.matmul(
                                prs[c][:],
                                w2s[:, i, :],
                                src[:, i, c * 512 : (c + 1) * 512],
                                start=(i == 0),
                                stop=(i == NT - 1),
                            )
                    return prs

                def transpose2(src2):
                    """[NO, BS] sbuf -> [128, NBLK*NO] psum (batch-major),
                    via regular identity matmuls: block.T @ I2 (exact fp32)."""
                    prt = prtp.tile([128, NBLK * NO], F32, tag="prt")
                    for blk in range(NBLK):
                        nc.tensor.matmul(
                            prt[:, blk * NO : (blk + 1) * NO],
                            src2[:, blk * 128 : (blk + 1) * 128],
                            ident[:NO, :NO],
                        )
                    return prt

                # ---- r_1 and q from cur1 ----
                prs = r_matmul(cur1)
                r_sb = workp.tile([NO, BS], F32, tag="rsb")
                q_sb = workp.tile([NO, BS], F32, tag="qsb")
                for c in range(NCH):
                    sl = slice(c * 512, (c + 1) * 512)
                    nc.vector.tensor_copy(r_sb[:, sl], prs[c][:])
                    nc.vector.tensor_scalar(
                        q_sb[:, sl], prs[c][:], b2c[:, 0:1], None, AOP.add
                    )
                pq = transpose2(q_sb)
                nc.vector.tensor_scalar(a_sb[:], pq[:], a_scale, None, AOP.mult)
                pr1 = transpose2(r_sb)
                nc.vector.tensor_add(rbuf[:, 1, :], pr1[:], a_sb[:])

                # ---- main loop: round k makes mem_k, r~_k, then outputs t=k-1 ----
                for k in range(2, T + 2):
                    src0 = cur1 if k == 2 else mem
                    for i in range(NT):
                        nc.vector._custom_dve(
                            lif,
                            out=mem[:, i, :],
                            in0=src0[:, i, :],
                            in1=cur1[:, i, :],
                            s0=BETA,
                            s1=THR,
                        )
                    prs = r_matmul(mem)
                    r_sb = workp.tile([NO, BS], F32, tag="rsb")
                    for c in range(NCH):
                        sl = slice(c * 512, (c + 1) * 512)
                        nc.vector.tensor_copy(r_sb[:, sl], prs[c][:])
                    prt = transpose2(r_sb)
                    nc.vector.tensor_add(rbuf[:, k, :], prt[:], a_sb[:])

                    t = k - 1  # emit step t outputs (1-based)
                    cur2 = workp.tile([128, NBLK * NO], F32, tag="cur2")
                    nc.vector.scalar_tensor_tensor(
                        cur2[:], rbuf[:, t, :], BETA, rbuf[:, k, :], AOP.mult,
                        AOP.subtract,
                    )
                    prev = zer32[:] if t == 1 else m2rec[:, t - 2, :]
                    nc.vector._custom_dve(
                        lif,
                        out=m2rec[:, t - 1, :],
                        in0=prev,
                        in1=cur2[:],
                        s0=BETA,
                        s1=THR,
                    )
                    nc.vector.tensor_scalar(
                        s2rec[:, t - 1, :], m2rec[:, t - 1, :], THR, None, AOP.is_gt
                    )
                    nc.sync.dma_start(
                        m2_d[t - 1, :, :].rearrange("(blk p) o -> p blk o", p=128),
                        m2rec[:, t - 1, :].rearrange("p (blk o) -> p blk o", o=NO),
                    )
                    nc.sync.dma_start(
                        s2_d[t - 1, :, :].rearrange("(blk p) o -> p blk o", p=128),
                        s2rec[:, t - 1, :].rearrange("p (blk o) -> p blk o", o=NO),
                    )
    nc.compile()
    return nc


_PROG = None


def _get_prog():
    global _PROG
    if _PROG is None:
        _PROG = _build_program()
    return _PROG


def _make_in_maps(x, W1, b1, W2, b2):
    x = np.ascontiguousarray(np.asarray(x, np.float32))
    W1 = np.asarray(W1, np.float32)
    b1 = np.asarray(b1, np.float32)
    W2 = np.asarray(W2, np.float32)
    b2 = np.asarray(b2, np.float32)

    w1t = np.zeros((NI, NHP), np.float32)
    w1t[:, :NH] = W1.T
    b1p = np.zeros((NHP,), np.float32)
    b1p[:NH] = b1
    b1c = b1p.reshape(NT, 128).T  # [128, NT]
    w2tp = np.zeros((NHP, NO), np.float32)
    w2tp[:NH] = W2.T
    w2s = w2tp.reshape(NT, 128, NO).transpose(1, 0, 2)  # [128, NT, NO]

    base = np.zeros((128, BLOBF), np.float32)
    base[:, OFF_ID : OFF_ID + 128] = np.eye(128, dtype=np.float32)
    base[:NI, OFF_W1 : OFF_W1 + NHP] = w1t
    base[:, OFF_B1 : OFF_B1 + NT] = b1c
    base[:, OFF_W2 : OFF_W2 + NT * NO] = w2s.reshape(128, NT * NO)
    base[:NO, OFF_B2] = b2

    xs = x.reshape(NCORES, NBLK, 128, NI)
    in_maps = []
    for i in range(NCORES):
        blob = base.copy()
        blob[:, OFF_X : OFF_X + NBLK * NI] = (
            xs[i].transpose(1, 0, 2).reshape(128, NBLK * NI)
        )
        in_maps.append({"blob": blob})
    return in_maps


def _run(x, W1, b1, W2, b2, **spmd_kwargs):
    nc = _get_prog()
    in_maps = _make_in_maps(x, W1, b1, W2, b2)
    res = run_bass_kernel_spmd(nc, in_maps, list(range(NCORES)), **spmd_kwargs)
    m2 = np.concatenate([res.results[i]["mem2_rec"] for i in range(NCORES)], axis=1)
    s2 = np.concatenate([res.results[i]["spk2_rec"] for i in range(NCORES)], axis=1)
    return (np.asarray(m2, np.float32), np.asarray(s2, np.float32)), res


def kernel(x, W1, b1, W2, b2):
    out, _ = _run(x, W1, b1, W2, b2)
    return out
